# revision 11
# baseline (speedup 1.0000x reference)
"""Channel attention (B=8, N=16384, C=512) Trainium2 Bass kernel.

Math (per batch b, with v = x^T [C, N]):
    energy  = v @ v^T                      [C, C]   (gram matrix, symmetric)
    att     = softmax(rowmax(e) - e)       == exp(rowmin(e) - e) / Z  (shift-invariant)
    out     = gamma * (att @ v) + v        [C, N]
    y       = out^T                        [N, C]

Sharding: data-parallel over B — core b computes batch b entirely.

Per-core dataflow (v2 — single fp16 energy matmul, no DRAM scratch):
  Phase 1 (stream x in 128-row chunks, cast to fp16 in-flight by SWDGE DMA):
    energy upper block-triangle += hk_bi^T @ hk_[bi:]  (one fp16 matmul per
    block; quantization error on the final output is ~6e-4, measured off-line)
    hk blocks are also transposed on the PE (regular matmul vs identity
    moving operand — much faster than transpose-mode) into a resident
    SBUF fp16 tensor hiT[c, n] (x^T), so phase 2 needs no DRAM round-trip.
  Interlude:
    mirror the triangle via PE transposes; rowmin; exp(min - e) with fused
    row-sum (ACT accum_out); W = I + gamma/Z * att^T in fp16 via PE matmuls.
    (W folds the softmax normalization, the gamma scale AND the residual.)
  Phase 2:
    y[n-chunk] = hiT-slice^T @ W  accumulated over 4 channel blocks
    (= x @ (I + gamma*att^T) = gamma*(att@v)^T + x, i.e. the final output).
"""

import sys

sys.path.insert(0, "/opt/trn_rl_repo")

from contextlib import ExitStack

import numpy as np

import concourse.bass as bass
import concourse.mybir as mybir
import concourse.tile as tile
from concourse import bacc
from concourse.bass_utils import run_bass_kernel_spmd
from concourse.masks import make_identity

B, N, C = 8, 16384, 512
P = 128
NK = N // P  # 128 row chunks
NB = C // P  # 4 channel blocks
F32 = mybir.dt.float32
F16 = mybir.dt.float16

_nc_cache = None


def _build():
    nc = bacc.Bacc()
    x_in = nc.dram_tensor("x", [N, C], F32, kind="ExternalInput")
    g_in = nc.dram_tensor("gamma", [1], F32, kind="ExternalInput")
    y_out = nc.dram_tensor("y", [N, C], F32, kind="ExternalOutput")

    with ExitStack() as ctx:
        tc = ctx.enter_context(tile.TileContext(nc))
        const = ctx.enter_context(tc.tile_pool(name="const", bufs=1))
        xpool = ctx.enter_context(tc.tile_pool(name="xpool", bufs=3))
        hpool = ctx.enter_context(tc.tile_pool(name="hpool", bufs=2))
        soft = ctx.enter_context(tc.tile_pool(name="soft", bufs=1))
        hiT_pool = ctx.enter_context(tc.tile_pool(name="hiT", bufs=1))
        opool = ctx.enter_context(tc.tile_pool(name="opool", bufs=2))
        psum_t_ctx = tc.tile_pool(name="psum_t", bufs=3, space="PSUM")
        psum_t = psum_t_ctx.__enter__()
        psum_e_ctx = tc.tile_pool(name="psum_e", bufs=1, space="PSUM")
        psum_e = psum_e_ctx.__enter__()

        ident16 = const.tile([P, P], F16)
        make_identity(nc, ident16)
        ident32 = const.tile([P, P], F32)
        make_identity(nc, ident32)
        gamma_sb = const.tile([P, 1], F32)
        nc.sync.dma_start(out=gamma_sb, in_=g_in[:].to_broadcast([P, 1]))

        x_ap = x_in[:]
        # [P, NK, C] views: partition = row-within-chunk, mid = chunk index
        x_v = x_ap.rearrange("(n p) c -> p n c", p=P)
        y_v = y_out[:].rearrange("(n p) c -> p n c", p=P)
        KB = 4  # k-chunks per phase-1 DMA group
        CB = 4  # n-chunks per phase-2 iteration

        # upper-triangle energy accumulators: row-block bi holds cols [bi*P, C)
        e_ps = [psum_e.tile([P, C - bi * P], F32, name=f"e{bi}", tag=f"e{bi}", bufs=1) for bi in range(NB)]
        # resident transposed hi: [c-within-block, block, n] fp16 (= x^T)
        hiT = hiT_pool.tile([P, NB, N], F16, name="hiT", tag="hiT")

        # ---------------- Phase 1: energy + on-chip transpose ----------------
        for kb in range(NK // KB):
            k0 = kb * KB
            xk = xpool.tile([P, KB, C], F32)
            # alternate HWDGE rings (SP / ACT) so DMA fixed costs pipeline
            dma_eng = nc.sync if kb % 2 == 0 else nc.scalar
            dma_eng.dma_start(out=xk, in_=x_v[:, k0 : k0 + KB, :])
            hk = hpool.tile([P, KB, C], F16)
            # fp16 round; alternate engines so neither becomes the bottleneck
            if kb % 2 == 0:
                nc.scalar.copy(out=hk, in_=xk)
            else:
                nc.vector.tensor_copy(hk, xk)

            for u in range(KB):
                k = k0 + u
                first = k == 0
                last = k == NK - 1
                tps = psum_t.tile([P, NB, P], F32, name="tps", tag="tps")
                for bi in range(NB):
                    j0 = bi * P
                    lhs = hk[:, u, j0 : j0 + P]
                    nc.tensor.matmul(e_ps[bi], lhs, hk[:, u, j0:C], start=first, stop=last)
                    # transpose of this block: regular matmul, identity moving
                    nc.tensor.matmul(tps[:, bi, :], lhs, ident16, start=True, stop=True)
                if k % 2 == 0:
                    nc.vector.tensor_copy(hiT[:, :, k * P : (k + 1) * P], tps)
                else:
                    nc.scalar.copy(out=hiT[:, :, k * P : (k + 1) * P], in_=tps)

        # ---------------- Interlude: softmax -> W = I + gamma * att^T ----------------
        e_row = [soft.tile([P, C], F32, name=f"erow{bi}", tag=f"erow{bi}") for bi in range(NB)]
        for bi in range(NB):
            nc.scalar.copy(out=e_row[bi][:, bi * P : C], in_=e_ps[bi])
        psum_e_ctx.__exit__(None, None, None)
        psum_t_ctx.__exit__(None, None, None)
        psum_m_ctx = tc.tile_pool(name="psum_m", bufs=6, space="PSUM")
        psum_m = psum_m_ctx.__enter__()
        # mirror the strict-lower blocks from the stored upper triangle
        for bi in range(NB):
            for bj in range(bi):
                pt = psum_m.tile([P, P], F32, tag="tp")
                nc.tensor.transpose(pt, e_row[bj][:, bi * P : (bi + 1) * P], ident32)
                nc.scalar.copy(out=e_row[bi][:, bj * P : (bj + 1) * P], in_=pt)

        W = [soft.tile([P, C], F16, name=f"W{bj}", tag=f"W{bj}") for bj in range(NB)]
        Bp = [soft.tile([P, C], F16, name=f"Bp{bi}", tag=f"Bp{bi}") for bi in range(NB)]
        for bi in range(NB):
            mn = soft.tile([P, 1], F32, tag=f"mn{bi}")
            nc.vector.tensor_reduce(
                out=mn, in_=e_row[bi], axis=mybir.AxisListType.X, op=mybir.AluOpType.min
            )
            bt = soft.tile([P, C], F32, tag=f"bt{bi}")
            zt = soft.tile([P, 1], F32, tag=f"zt{bi}")
            nc.scalar.activation(
                out=bt,
                in_=e_row[bi],
                func=mybir.ActivationFunctionType.Exp,
                bias=mn,
                scale=-1.0,
                accum_out=zt,
            )
            rz = soft.tile([P, 1], F32, tag=f"rz{bi}")
            nc.vector.reciprocal(out=rz, in_=zt)
            gr = soft.tile([P, 1], F32, tag=f"gr{bi}")
            nc.vector.tensor_mul(gr, rz, gamma_sb)
            nc.vector.tensor_scalar_mul(Bp[bi], bt, gr)  # fp16: gamma*att rows
        # W[bj][:, bi*P:(bi+1)*P] = Bp[bi][:, bj*P:(bj+1)*P]^T via regular matmuls
        for bi in range(NB):
            for bj in range(NB):
                pw = psum_m.tile([P, P], F32, name="pw", tag="tp")
                nc.tensor.matmul(pw, Bp[bi][:, bj * P : (bj + 1) * P], ident16, start=True, stop=True)
                nc.vector.tensor_copy(W[bj][:, bi * P : (bi + 1) * P], pw)
        for bj in range(NB):
            nc.vector.tensor_add(
                W[bj][:, bj * P : (bj + 1) * P], W[bj][:, bj * P : (bj + 1) * P], ident16
            )

        psum_m_ctx.__exit__(None, None, None)
        psum = ctx.enter_context(tc.tile_pool(name="psum", bufs=2, space="PSUM"))

        # ---------------- Phase 2: y = x @ W ----------------
        for cb in range(NK // CB):
            c0 = cb * CB
            ops = psum.tile([P, CB, C], F32, tag="ops", bufs=2)
            for u in range(CB):
                r0 = (c0 + u) * P
                for bj in range(NB):
                    nc.tensor.matmul(
                        ops[:, u, :],
                        hiT[:, bj, r0 : r0 + P],
                        W[bj],
                        start=(bj == 0),
                        stop=(bj == NB - 1),
                    )
            ob = opool.tile([P, CB, C], F32)
            # split the PSUM drain between ACT and DVE so it hides under the MMs
            nc.scalar.copy(out=ob[:, 0 : CB // 2, :], in_=ops[:, 0 : CB // 2, :])
            nc.vector.tensor_copy(ob[:, CB // 2 : CB, :], ops[:, CB // 2 : CB, :])
            nc.sync.dma_start(out=y_v[:, c0 : c0 + CB, :], in_=ob)

    nc.finalize()
    return nc


def _get_nc():
    global _nc_cache
    if _nc_cache is None:
        _nc_cache = _build()
    return _nc_cache


def kernel(x, gamma, _trace=False):
    x = np.ascontiguousarray(np.asarray(x), dtype=np.float32)
    gamma = np.ascontiguousarray(np.asarray(gamma), dtype=np.float32)
    nc = _get_nc()
    in_maps = [
        {"x": np.ascontiguousarray(x[b]), "gamma": gamma} for b in range(B)
    ]
    res = run_bass_kernel_spmd(nc, in_maps, list(range(B)), trace=_trace)
    out = np.stack([r["y"] for r in res.results], axis=0)
    if _trace:
        return out, res
    return out


# revision 14
# speedup vs baseline: 1.0151x; 1.0151x over previous
"""Channel attention (B=8, N=16384, C=512) Trainium2 Bass kernel.

Math (per batch b, with v = x^T [C, N]):
    energy  = v @ v^T                      [C, C]   (gram matrix, symmetric)
    att     = softmax(rowmax(e) - e)       == exp(rowmin(e) - e) / Z  (shift-invariant)
    out     = gamma * (att @ v) + v        [C, N]
    y       = out^T                        [N, C]

Sharding: data-parallel over B — core b computes batch b entirely.

Per-core dataflow (v2 — single fp16 energy matmul, no DRAM scratch):
  Phase 1 (stream x in 128-row chunks, cast to fp16 in-flight by SWDGE DMA):
    energy upper block-triangle += hk_bi^T @ hk_[bi:]  (one fp16 matmul per
    block; quantization error on the final output is ~6e-4, measured off-line)
    hk blocks are also transposed on the PE (regular matmul vs identity
    moving operand — much faster than transpose-mode) into a resident
    SBUF fp16 tensor hiT[c, n] (x^T), so phase 2 needs no DRAM round-trip.
  Interlude:
    mirror the triangle via PE transposes; rowmin; exp(min - e) with fused
    row-sum (ACT accum_out); W = I + gamma/Z * att^T in fp16 via PE matmuls.
    (W folds the softmax normalization, the gamma scale AND the residual.)
  Phase 2:
    y[n-chunk] = hiT-slice^T @ W  accumulated over 4 channel blocks
    (= x @ (I + gamma*att^T) = gamma*(att@v)^T + x, i.e. the final output).
"""

import sys

sys.path.insert(0, "/opt/trn_rl_repo")

from contextlib import ExitStack

import numpy as np

import concourse.bass as bass
import concourse.mybir as mybir
import concourse.tile as tile
from concourse import bacc
from concourse.bass_utils import run_bass_kernel_spmd
from concourse.masks import make_identity

B, N, C = 8, 16384, 512
P = 128
NK = N // P  # 128 row chunks
NB = C // P  # 4 channel blocks
F32 = mybir.dt.float32
F16 = mybir.dt.float16

_nc_cache = None


def _build():
    nc = bacc.Bacc()
    x_in = nc.dram_tensor("x", [N, C], F32, kind="ExternalInput")
    g_in = nc.dram_tensor("gamma", [1], F32, kind="ExternalInput")
    y_out = nc.dram_tensor("y", [N, C], F32, kind="ExternalOutput")

    with ExitStack() as ctx:
        tc = ctx.enter_context(tile.TileContext(nc))
        const = ctx.enter_context(tc.tile_pool(name="const", bufs=1))
        xpool = ctx.enter_context(tc.tile_pool(name="xpool", bufs=3))
        hpool = ctx.enter_context(tc.tile_pool(name="hpool", bufs=2))
        soft = ctx.enter_context(tc.tile_pool(name="soft", bufs=1))
        hiT_pool = ctx.enter_context(tc.tile_pool(name="hiT", bufs=1))
        opool = ctx.enter_context(tc.tile_pool(name="opool", bufs=2))
        psum_t_ctx = tc.tile_pool(name="psum_t", bufs=3, space="PSUM")
        psum_t = psum_t_ctx.__enter__()
        psum_e_ctx = tc.tile_pool(name="psum_e", bufs=1, space="PSUM")
        psum_e = psum_e_ctx.__enter__()

        ident16 = const.tile([P, P], F16)
        make_identity(nc, ident16)
        ident32 = const.tile([P, P], F32)
        make_identity(nc, ident32)
        gamma_sb = const.tile([P, 1], F32)
        nc.sync.dma_start(out=gamma_sb, in_=g_in[:].to_broadcast([P, 1]))

        KB = 4  # k-chunks per phase-1 DMA group
        CB = 4  # n-chunks per phase-2 iteration
        # row n = g*512 + p*4 + u: each partition owns 4 CONSECUTIVE rows of a
        # 512-row group -> 8 KiB contiguous DRAM per partition per DMA (vs 2 KiB
        # with the row-major chunk mapping) => ~340 GB/s instead of ~250.
        # The row permutation is globally consistent: energy sums all rows
        # (order-free), hiT columns are indexed by (g,u)-chunk, and phase 2
        # emits the same permuted rows it reads from hiT.
        x_v = x_in[:].rearrange("(g p u) c -> p g u c", p=P, u=KB)
        y_v = y_out[:].rearrange("(g p u) c -> p g u c", p=P, u=CB)

        # upper-triangle energy accumulators: row-block bi holds cols [bi*P, C)
        e_ps = [psum_e.tile([P, C - bi * P], F32, name=f"e{bi}", tag=f"e{bi}", bufs=1) for bi in range(NB)]
        # resident transposed hi: [c-within-block, block, n] fp16 (= x^T)
        hiT = hiT_pool.tile([P, NB, N], F16, name="hiT", tag="hiT")

        # ---------------- Phase 1: energy + on-chip transpose ----------------
        for kb in range(NK // KB):
            k0 = kb * KB
            xk = xpool.tile([P, KB, C], F32)
            # alternate HWDGE rings (SP / ACT) so DMA fixed costs pipeline
            dma_eng = nc.sync if kb % 2 == 0 else nc.scalar
            dma_eng.dma_start(out=xk, in_=x_v[:, kb])
            hk = hpool.tile([P, KB, C], F16)
            # fp16 round; alternate engines so neither becomes the bottleneck
            if kb % 2 == 0:
                nc.scalar.copy(out=hk, in_=xk)
            else:
                nc.vector.tensor_copy(hk, xk)

            for u in range(KB):
                k = k0 + u
                first = k == 0
                last = k == NK - 1
                tps = psum_t.tile([P, NB, P], F32, name="tps", tag="tps")
                for bi in range(NB):
                    j0 = bi * P
                    lhs = hk[:, u, j0 : j0 + P]
                    nc.tensor.matmul(e_ps[bi], lhs, hk[:, u, j0:C], start=first, stop=last)
                    # transpose of this block: regular matmul, identity moving
                    nc.tensor.matmul(tps[:, bi, :], lhs, ident16, start=True, stop=True)
                if k % 2 == 0:
                    nc.vector.tensor_copy(hiT[:, :, k * P : (k + 1) * P], tps)
                else:
                    nc.scalar.copy(out=hiT[:, :, k * P : (k + 1) * P], in_=tps)

        # ---------------- Interlude: softmax -> W = I + gamma * att^T ----------------
        e_row = [soft.tile([P, C], F32, name=f"erow{bi}", tag=f"erow{bi}") for bi in range(NB)]
        for bi in range(NB):
            nc.scalar.copy(out=e_row[bi][:, bi * P : C], in_=e_ps[bi])
        psum_e_ctx.__exit__(None, None, None)
        psum_t_ctx.__exit__(None, None, None)
        psum_m_ctx = tc.tile_pool(name="psum_m", bufs=6, space="PSUM")
        psum_m = psum_m_ctx.__enter__()
        # mirror the strict-lower blocks from the stored upper triangle
        for bi in range(NB):
            for bj in range(bi):
                pt = psum_m.tile([P, P], F32, tag="tp")
                nc.tensor.transpose(pt, e_row[bj][:, bi * P : (bi + 1) * P], ident32)
                nc.scalar.copy(out=e_row[bi][:, bj * P : (bj + 1) * P], in_=pt)

        W = [soft.tile([P, C], F16, name=f"W{bj}", tag=f"W{bj}") for bj in range(NB)]
        Bp = [soft.tile([P, C], F16, name=f"Bp{bi}", tag=f"Bp{bi}") for bi in range(NB)]
        for bi in range(NB):
            mn = soft.tile([P, 1], F32, tag=f"mn{bi}")
            nc.vector.tensor_reduce(
                out=mn, in_=e_row[bi], axis=mybir.AxisListType.X, op=mybir.AluOpType.min
            )
            bt = soft.tile([P, C], F32, tag=f"bt{bi}")
            zt = soft.tile([P, 1], F32, tag=f"zt{bi}")
            nc.scalar.activation(
                out=bt,
                in_=e_row[bi],
                func=mybir.ActivationFunctionType.Exp,
                bias=mn,
                scale=-1.0,
                accum_out=zt,
            )
            rz = soft.tile([P, 1], F32, tag=f"rz{bi}")
            nc.vector.reciprocal(out=rz, in_=zt)
            gr = soft.tile([P, 1], F32, tag=f"gr{bi}")
            nc.vector.tensor_mul(gr, rz, gamma_sb)
            nc.vector.tensor_scalar_mul(Bp[bi], bt, gr)  # fp16: gamma*att rows
        # W[bj][:, bi*P:(bi+1)*P] = Bp[bi][:, bj*P:(bj+1)*P]^T via regular matmuls
        for bi in range(NB):
            for bj in range(NB):
                pw = psum_m.tile([P, P], F32, name="pw", tag="tp")
                nc.tensor.matmul(pw, Bp[bi][:, bj * P : (bj + 1) * P], ident16, start=True, stop=True)
                nc.vector.tensor_copy(W[bj][:, bi * P : (bi + 1) * P], pw)
        for bj in range(NB):
            nc.vector.tensor_add(
                W[bj][:, bj * P : (bj + 1) * P], W[bj][:, bj * P : (bj + 1) * P], ident16
            )

        psum_m_ctx.__exit__(None, None, None)
        psum = ctx.enter_context(tc.tile_pool(name="psum", bufs=2, space="PSUM"))

        # ---------------- Phase 2: y = x @ W ----------------
        for cb in range(NK // CB):
            c0 = cb * CB
            ops = psum.tile([P, CB, C], F32, tag="ops", bufs=2)
            for u in range(CB):
                r0 = (c0 + u) * P
                for bj in range(NB):
                    nc.tensor.matmul(
                        ops[:, u, :],
                        hiT[:, bj, r0 : r0 + P],
                        W[bj],
                        start=(bj == 0),
                        stop=(bj == NB - 1),
                    )
            ob = opool.tile([P, CB, C], F32)
            # split the PSUM drain between ACT and DVE so it hides under the MMs
            nc.scalar.copy(out=ob[:, 0 : CB // 2, :], in_=ops[:, 0 : CB // 2, :])
            nc.vector.tensor_copy(ob[:, CB // 2 : CB, :], ops[:, CB // 2 : CB, :])
            y_eng = nc.sync if cb % 2 == 0 else nc.scalar
            y_eng.dma_start(out=y_v[:, cb], in_=ob)

    nc.finalize()
    return nc


def _get_nc():
    global _nc_cache
    if _nc_cache is None:
        _nc_cache = _build()
    return _nc_cache


def kernel(x, gamma, _trace=False):
    x = np.ascontiguousarray(np.asarray(x), dtype=np.float32)
    gamma = np.ascontiguousarray(np.asarray(gamma), dtype=np.float32)
    nc = _get_nc()
    in_maps = [
        {"x": np.ascontiguousarray(x[b]), "gamma": gamma} for b in range(B)
    ]
    res = run_bass_kernel_spmd(nc, in_maps, list(range(B)), trace=_trace)
    out = np.stack([r["y"] for r in res.results], axis=0)
    if _trace:
        return out, res
    return out


# revision 18
# speedup vs baseline: 1.0165x; 1.0014x over previous
"""Channel attention (B=8, N=16384, C=512) Trainium2 Bass kernel.

Math (per batch b, with v = x^T [C, N]):
    energy  = v @ v^T                      [C, C]   (gram matrix, symmetric)
    att     = softmax(rowmax(e) - e)       == exp(rowmin(e) - e) / Z  (shift-invariant)
    out     = gamma * (att @ v) + v        [C, N]
    y       = out^T                        [N, C]

Sharding: data-parallel over B — core b computes batch b entirely.

Per-core dataflow (v2 — single fp16 energy matmul, no DRAM scratch):
  Phase 1 (stream x in 128-row chunks, cast to fp16 in-flight by SWDGE DMA):
    energy upper block-triangle += hk_bi^T @ hk_[bi:]  (one fp16 matmul per
    block; quantization error on the final output is ~6e-4, measured off-line)
    hk blocks are also transposed on the PE (regular matmul vs identity
    moving operand — much faster than transpose-mode) into a resident
    SBUF fp16 tensor hiT[c, n] (x^T), so phase 2 needs no DRAM round-trip.
  Interlude:
    mirror the triangle via PE transposes; rowmin; exp(min - e) with fused
    row-sum (ACT accum_out); W = I + gamma/Z * att^T in fp16 via PE matmuls.
    (W folds the softmax normalization, the gamma scale AND the residual.)
  Phase 2:
    y[n-chunk] = hiT-slice^T @ W  accumulated over 4 channel blocks
    (= x @ (I + gamma*att^T) = gamma*(att@v)^T + x, i.e. the final output).
"""

import sys

sys.path.insert(0, "/opt/trn_rl_repo")

from contextlib import ExitStack

import numpy as np

import concourse.bass as bass
import concourse.mybir as mybir
import concourse.tile as tile
from concourse import bacc
from concourse.bass_utils import run_bass_kernel_spmd
from concourse.masks import make_identity

B, N, C = 8, 16384, 512
P = 128
NK = N // P  # 128 row chunks
NB = C // P  # 4 channel blocks
F32 = mybir.dt.float32
F16 = mybir.dt.float16

_nc_cache = None


def _build():
    nc = bacc.Bacc()
    x_in = nc.dram_tensor("x", [N, C], F32, kind="ExternalInput")
    g_in = nc.dram_tensor("gamma", [1], F32, kind="ExternalInput")
    y_out = nc.dram_tensor("y", [N, C], F32, kind="ExternalOutput")

    with ExitStack() as ctx:
        tc = ctx.enter_context(tile.TileContext(nc))
        const = ctx.enter_context(tc.tile_pool(name="const", bufs=1))
        xpool = ctx.enter_context(tc.tile_pool(name="xpool", bufs=3))
        hpool = ctx.enter_context(tc.tile_pool(name="hpool", bufs=2))
        soft = ctx.enter_context(tc.tile_pool(name="soft", bufs=1))
        hiT_pool = ctx.enter_context(tc.tile_pool(name="hiT", bufs=1))
        opool = ctx.enter_context(tc.tile_pool(name="opool", bufs=2))
        psum_t_ctx = tc.tile_pool(name="psum_t", bufs=3, space="PSUM")
        psum_t = psum_t_ctx.__enter__()
        psum_e_ctx = tc.tile_pool(name="psum_e", bufs=1, space="PSUM")
        psum_e = psum_e_ctx.__enter__()

        ident16 = const.tile([P, P], F16)
        make_identity(nc, ident16)
        ident32 = const.tile([P, P], F32)
        make_identity(nc, ident32)
        gamma_sb = const.tile([P, 1], F32)
        nc.sync.dma_start(out=gamma_sb, in_=g_in[:].to_broadcast([P, 1]))

        KB = 4  # k-chunks per phase-1 DMA group
        CB = 4  # n-chunks per phase-2 iteration
        # row n = g*512 + p*4 + u: each partition owns 4 CONSECUTIVE rows of a
        # 512-row group -> 8 KiB contiguous DRAM per partition per DMA (vs 2 KiB
        # with the row-major chunk mapping) => ~340 GB/s instead of ~250.
        # The row permutation is globally consistent: energy sums all rows
        # (order-free), hiT columns are indexed by (g,u)-chunk, and phase 2
        # emits the same permuted rows it reads from hiT.
        x_v = x_in[:].rearrange("(g p u) c -> p g u c", p=P, u=KB)
        y_v = y_out[:].rearrange("(g p u) c -> p g u c", p=P, u=CB)

        # upper-triangle energy accumulators: row-block bi holds cols [bi*P, C)
        e_ps = [psum_e.tile([P, C - bi * P], F32, name=f"e{bi}", tag=f"e{bi}", bufs=1) for bi in range(NB)]
        # resident transposed hi: [c-within-block, block, n] fp16 (= x^T)
        hiT = hiT_pool.tile([P, NB, N], F16, name="hiT", tag="hiT")

        # ---------------- Phase 1: energy + on-chip transpose ----------------
        for kb in range(NK // KB):
            k0 = kb * KB
            xk = xpool.tile([P, KB, C], F32)
            # alternate HWDGE rings (SP / ACT) so DMA fixed costs pipeline
            dma_eng = nc.sync if kb % 2 == 0 else nc.scalar
            dma_eng.dma_start(out=xk, in_=x_v[:, kb])
            hk = hpool.tile([P, KB, C], F16)
            # fp16 round; alternate engines so neither becomes the bottleneck
            if kb % 2 == 0:
                nc.scalar.copy(out=hk, in_=xk)
            else:
                nc.vector.tensor_copy(hk, xk)

            for u in range(KB):
                k = k0 + u
                first = k == 0
                last = k == NK - 1
                tps = psum_t.tile([P, NB, P], F32, name="tps", tag="tps")
                for bi in range(NB):
                    j0 = bi * P
                    lhs = hk[:, u, j0 : j0 + P]
                    nc.tensor.matmul(e_ps[bi], lhs, hk[:, u, j0:C], start=first, stop=last)
                    # transpose of this block: regular matmul, identity moving
                    nc.tensor.matmul(tps[:, bi, :], lhs, ident16, start=True, stop=True)
                if k % 2 == 0:
                    nc.vector.tensor_copy(hiT[:, :, k * P : (k + 1) * P], tps)
                else:
                    nc.scalar.copy(out=hiT[:, :, k * P : (k + 1) * P], in_=tps)

        # ---------------- Interlude: softmax -> W = I + gamma * att^T ----------------
        e_row = [soft.tile([P, C], F32, name=f"erow{bi}", tag=f"erow{bi}") for bi in range(NB)]
        for bi in range(NB):
            if bi % 2 == 0:
                nc.scalar.copy(out=e_row[bi][:, bi * P : C], in_=e_ps[bi])
            else:
                nc.vector.tensor_copy(e_row[bi][:, bi * P : C], e_ps[bi])
        psum_e_ctx.__exit__(None, None, None)
        psum_t_ctx.__exit__(None, None, None)
        psum_m_ctx = tc.tile_pool(name="psum_m", bufs=6, space="PSUM")
        psum_m = psum_m_ctx.__enter__()
        # mirror the strict-lower blocks from the stored upper triangle
        nm = 0
        for bi in range(NB):
            for bj in range(bi):
                pt = psum_m.tile([P, P], F32, tag="tp")
                nc.tensor.transpose(pt, e_row[bj][:, bi * P : (bi + 1) * P], ident32)
                if nm % 2 == 0:
                    nc.vector.tensor_copy(e_row[bi][:, bj * P : (bj + 1) * P], pt)
                else:
                    nc.scalar.copy(out=e_row[bi][:, bj * P : (bj + 1) * P], in_=pt)
                nm += 1

        W = [soft.tile([P, C], F16, name=f"W{bj}", tag=f"W{bj}") for bj in range(NB)]
        Bp = [soft.tile([P, C], F16, name=f"Bp{bi}", tag=f"Bp{bi}") for bi in range(NB)]
        for bi in range(NB):
            mn = soft.tile([P, 1], F32, tag=f"mn{bi}")
            nc.vector.tensor_reduce(
                out=mn, in_=e_row[bi], axis=mybir.AxisListType.X, op=mybir.AluOpType.min
            )
            bt = soft.tile([P, C], F32, tag=f"bt{bi}")
            zt = soft.tile([P, 1], F32, tag=f"zt{bi}")
            nc.scalar.activation(
                out=bt,
                in_=e_row[bi],
                func=mybir.ActivationFunctionType.Exp,
                bias=mn,
                scale=-1.0,
                accum_out=zt,
            )
            rz = soft.tile([P, 1], F32, tag=f"rz{bi}")
            nc.vector.reciprocal(out=rz, in_=zt)
            gr = soft.tile([P, 1], F32, tag=f"gr{bi}")
            nc.vector.tensor_mul(gr, rz, gamma_sb)
            nc.vector.tensor_scalar_mul(Bp[bi], bt, gr)  # fp16: gamma*att rows
        # W[bj][:, bi*P:(bi+1)*P] = Bp[bi][:, bj*P:(bj+1)*P]^T via regular matmuls
        for bi in range(NB):
            for bj in range(NB):
                pw = psum_m.tile([P, P], F32, name="pw", tag="tp")
                nc.tensor.matmul(pw, Bp[bi][:, bj * P : (bj + 1) * P], ident16, start=True, stop=True)
                if (bi + bj) % 2 == 0:
                    nc.vector.tensor_copy(W[bj][:, bi * P : (bi + 1) * P], pw)
                else:
                    nc.scalar.copy(out=W[bj][:, bi * P : (bi + 1) * P], in_=pw)
        for bj in range(NB):
            nc.vector.tensor_add(
                W[bj][:, bj * P : (bj + 1) * P], W[bj][:, bj * P : (bj + 1) * P], ident16
            )

        psum_m_ctx.__exit__(None, None, None)
        psum = ctx.enter_context(tc.tile_pool(name="psum", bufs=2, space="PSUM"))

        # ---------------- Phase 2: y = x @ W ----------------
        for cb in range(NK // CB):
            c0 = cb * CB
            ops = psum.tile([P, CB, C], F32, tag="ops", bufs=2)
            for u in range(CB):
                r0 = (c0 + u) * P
                for bj in range(NB):
                    nc.tensor.matmul(
                        ops[:, u, :],
                        hiT[:, bj, r0 : r0 + P],
                        W[bj],
                        start=(bj == 0),
                        stop=(bj == NB - 1),
                    )
            ob = opool.tile([P, CB, C], F32)
            # per-chunk PSUM drain split between ACT and DVE: each chunk's copy
            # fires right after its accumulation stops, so the y DMA starts
            # ~one chunk after the group's last matmul
            for u in range(CB):
                if u % 2 == 0:
                    nc.scalar.copy(out=ob[:, u, :], in_=ops[:, u, :])
                else:
                    nc.vector.tensor_copy(ob[:, u, :], ops[:, u, :])
            y_eng = nc.sync if cb % 2 == 0 else nc.scalar
            y_eng.dma_start(out=y_v[:, cb], in_=ob)

    nc.finalize()
    return nc


def _get_nc():
    global _nc_cache
    if _nc_cache is None:
        _nc_cache = _build()
    return _nc_cache


def kernel(x, gamma, _trace=False):
    x = np.ascontiguousarray(np.asarray(x), dtype=np.float32)
    gamma = np.ascontiguousarray(np.asarray(gamma), dtype=np.float32)
    nc = _get_nc()
    in_maps = [
        {"x": np.ascontiguousarray(x[b]), "gamma": gamma} for b in range(B)
    ]
    res = run_bass_kernel_spmd(nc, in_maps, list(range(B)), trace=_trace)
    out = np.stack([r["y"] for r in res.results], axis=0)
    if _trace:
        return out, res
    return out


# revision 20
# speedup vs baseline: 1.1043x; 1.0863x over previous
"""Channel attention (B=8, N=16384, C=512) Trainium2 Bass kernel.

Math (per batch b, with v = x^T [C, N]):
    energy  = v @ v^T                      [C, C]   (gram matrix, symmetric)
    att     = softmax(rowmax(e) - e)       == exp(rowmin(e) - e) / Z  (shift-invariant)
    out     = gamma * (att @ v) + v        [C, N]
    y       = out^T                        [N, C]

Sharding: data-parallel over B — core b computes batch b entirely.

Per-core dataflow (v2 — single fp16 energy matmul, no DRAM scratch):
  Phase 1 (stream x in 128-row chunks, cast to fp16 in-flight by SWDGE DMA):
    energy upper block-triangle += hk_bi^T @ hk_[bi:]  (one fp16 matmul per
    block; quantization error on the final output is ~6e-4, measured off-line)
    hk blocks are also transposed on the PE (regular matmul vs identity
    moving operand — much faster than transpose-mode) into a resident
    SBUF fp16 tensor hiT[c, n] (x^T), so phase 2 needs no DRAM round-trip.
  Interlude:
    mirror the triangle via PE transposes; rowmin; exp(min - e) with fused
    row-sum (ACT accum_out); W = I + gamma/Z * att^T in fp16 via PE matmuls.
    (W folds the softmax normalization, the gamma scale AND the residual.)
  Phase 2:
    y[n-chunk] = hiT-slice^T @ W  accumulated over 4 channel blocks
    (= x @ (I + gamma*att^T) = gamma*(att@v)^T + x, i.e. the final output).
"""

import sys

sys.path.insert(0, "/opt/trn_rl_repo")

from contextlib import ExitStack

import numpy as np

import concourse.bass as bass
import concourse.mybir as mybir
import concourse.tile as tile
from concourse import bacc
from concourse.bass_utils import run_bass_kernel_spmd
from concourse.masks import make_identity

B, N, C = 8, 16384, 512
P = 128
NK = N // P  # 128 row chunks
NB = C // P  # 4 channel blocks
F32 = mybir.dt.float32
F16 = mybir.dt.float16

_nc_cache = None


def _build():
    nc = bacc.Bacc()
    x_in = nc.dram_tensor("x", [N, C], F32, kind="ExternalInput")
    g_in = nc.dram_tensor("gamma", [1], F32, kind="ExternalInput")
    y_out = nc.dram_tensor("y", [N, C], F32, kind="ExternalOutput")

    with ExitStack() as ctx:
        tc = ctx.enter_context(tile.TileContext(nc))
        const = ctx.enter_context(tc.tile_pool(name="const", bufs=1))
        hpool = ctx.enter_context(tc.tile_pool(name="hpool", bufs=4))
        soft = ctx.enter_context(tc.tile_pool(name="soft", bufs=1))
        hiT_pool = ctx.enter_context(tc.tile_pool(name="hiT", bufs=1))
        opool = ctx.enter_context(tc.tile_pool(name="opool", bufs=2))
        psum_t_ctx = tc.tile_pool(name="psum_t", bufs=3, space="PSUM")
        psum_t = psum_t_ctx.__enter__()
        psum_e_ctx = tc.tile_pool(name="psum_e", bufs=1, space="PSUM")
        psum_e = psum_e_ctx.__enter__()

        ident16 = const.tile([P, P], F16)
        make_identity(nc, ident16)
        ident32 = const.tile([P, P], F32)
        make_identity(nc, ident32)
        gamma_sb = const.tile([P, 1], F32)
        nc.sync.dma_start(out=gamma_sb, in_=g_in[:].to_broadcast([P, 1]))

        KB = 4  # k-chunks per phase-1 DMA group
        CB = 4  # n-chunks per phase-2 iteration
        # row n = g*512 + p*4 + u: each partition owns 4 CONSECUTIVE rows of a
        # 512-row group -> 8 KiB contiguous DRAM per partition per DMA (vs 2 KiB
        # with the row-major chunk mapping) => ~340 GB/s instead of ~250.
        # The row permutation is globally consistent: energy sums all rows
        # (order-free), hiT columns are indexed by (g,u)-chunk, and phase 2
        # emits the same permuted rows it reads from hiT.
        x_v = x_in[:].rearrange("(g p u) c -> p g u c", p=P, u=KB)
        y_v = y_out[:].rearrange("(g p u) c -> p g u c", p=P, u=CB)

        # upper-triangle energy accumulators: row-block bi holds cols [bi*P, C)
        e_ps = [psum_e.tile([P, C - bi * P], F32, name=f"e{bi}", tag=f"e{bi}", bufs=1) for bi in range(NB)]
        # resident transposed hi: [c-within-block, block, n] fp16 (= x^T)
        hiT = hiT_pool.tile([P, NB, N], F16, name="hiT", tag="hiT")

        # ---------------- Phase 1: energy + on-chip transpose ----------------
        for kb in range(NK // KB):
            k0 = kb * KB
            hk = hpool.tile([P, KB, C], F16)
            # SWDGE cast-DMA: fp32 DRAM -> fp16 SBUF in flight (8 KiB-contig
            # reads / 4 KiB-contig writes per partition with the permuted view)
            nc.gpsimd.dma_start(out=hk, in_=x_v[:, kb])

            for u in range(KB):
                k = k0 + u
                first = k == 0
                last = k == NK - 1
                tps = psum_t.tile([P, NB, P], F32, name="tps", tag="tps")
                for bi in range(NB):
                    j0 = bi * P
                    lhs = hk[:, u, j0 : j0 + P]
                    nc.tensor.matmul(e_ps[bi], lhs, hk[:, u, j0:C], start=first, stop=last)
                    # transpose of this block: regular matmul, identity moving
                    nc.tensor.matmul(tps[:, bi, :], lhs, ident16, start=True, stop=True)
                if k % 2 == 0:
                    nc.vector.tensor_copy(hiT[:, :, k * P : (k + 1) * P], tps)
                else:
                    nc.scalar.copy(out=hiT[:, :, k * P : (k + 1) * P], in_=tps)

        # ---------------- Interlude: softmax -> W = I + gamma * att^T ----------------
        e_row = [soft.tile([P, C], F32, name=f"erow{bi}", tag=f"erow{bi}") for bi in range(NB)]
        for bi in range(NB):
            if bi % 2 == 0:
                nc.scalar.copy(out=e_row[bi][:, bi * P : C], in_=e_ps[bi])
            else:
                nc.vector.tensor_copy(e_row[bi][:, bi * P : C], e_ps[bi])
        psum_e_ctx.__exit__(None, None, None)
        psum_t_ctx.__exit__(None, None, None)
        psum_m_ctx = tc.tile_pool(name="psum_m", bufs=6, space="PSUM")
        psum_m = psum_m_ctx.__enter__()
        # mirror the strict-lower blocks from the stored upper triangle
        nm = 0
        for bi in range(NB):
            for bj in range(bi):
                pt = psum_m.tile([P, P], F32, tag="tp")
                nc.tensor.transpose(pt, e_row[bj][:, bi * P : (bi + 1) * P], ident32)
                if nm % 2 == 0:
                    nc.vector.tensor_copy(e_row[bi][:, bj * P : (bj + 1) * P], pt)
                else:
                    nc.scalar.copy(out=e_row[bi][:, bj * P : (bj + 1) * P], in_=pt)
                nm += 1

        W = [soft.tile([P, C], F16, name=f"W{bj}", tag=f"W{bj}") for bj in range(NB)]
        Bp = [soft.tile([P, C], F16, name=f"Bp{bi}", tag=f"Bp{bi}") for bi in range(NB)]
        for bi in range(NB):
            mn = soft.tile([P, 1], F32, tag=f"mn{bi}")
            nc.vector.tensor_reduce(
                out=mn, in_=e_row[bi], axis=mybir.AxisListType.X, op=mybir.AluOpType.min
            )
            bt = soft.tile([P, C], F32, tag=f"bt{bi}")
            zt = soft.tile([P, 1], F32, tag=f"zt{bi}")
            nc.scalar.activation(
                out=bt,
                in_=e_row[bi],
                func=mybir.ActivationFunctionType.Exp,
                bias=mn,
                scale=-1.0,
                accum_out=zt,
            )
            rz = soft.tile([P, 1], F32, tag=f"rz{bi}")
            nc.vector.reciprocal(out=rz, in_=zt)
            gr = soft.tile([P, 1], F32, tag=f"gr{bi}")
            nc.vector.tensor_mul(gr, rz, gamma_sb)
            nc.vector.tensor_scalar_mul(Bp[bi], bt, gr)  # fp16: gamma*att rows
        # W[bj][:, bi*P:(bi+1)*P] = Bp[bi][:, bj*P:(bj+1)*P]^T via regular matmuls
        for bi in range(NB):
            for bj in range(NB):
                pw = psum_m.tile([P, P], F32, name="pw", tag="tp")
                nc.tensor.matmul(pw, Bp[bi][:, bj * P : (bj + 1) * P], ident16, start=True, stop=True)
                if (bi + bj) % 2 == 0:
                    nc.vector.tensor_copy(W[bj][:, bi * P : (bi + 1) * P], pw)
                else:
                    nc.scalar.copy(out=W[bj][:, bi * P : (bi + 1) * P], in_=pw)
        for bj in range(NB):
            nc.vector.tensor_add(
                W[bj][:, bj * P : (bj + 1) * P], W[bj][:, bj * P : (bj + 1) * P], ident16
            )

        psum_m_ctx.__exit__(None, None, None)
        psum = ctx.enter_context(tc.tile_pool(name="psum", bufs=2, space="PSUM"))

        # ---------------- Phase 2: y = x @ W ----------------
        for cb in range(NK // CB):
            c0 = cb * CB
            ops = psum.tile([P, CB, C], F32, tag="ops", bufs=2)
            for u in range(CB):
                r0 = (c0 + u) * P
                for bj in range(NB):
                    nc.tensor.matmul(
                        ops[:, u, :],
                        hiT[:, bj, r0 : r0 + P],
                        W[bj],
                        start=(bj == 0),
                        stop=(bj == NB - 1),
                    )
            ob = opool.tile([P, CB, C], F32)
            # per-chunk PSUM drain split between ACT and DVE: each chunk's copy
            # fires right after its accumulation stops, so the y DMA starts
            # ~one chunk after the group's last matmul
            for u in range(CB):
                if u % 2 == 0:
                    nc.scalar.copy(out=ob[:, u, :], in_=ops[:, u, :])
                else:
                    nc.vector.tensor_copy(ob[:, u, :], ops[:, u, :])
            y_eng = nc.sync if cb % 2 == 0 else nc.scalar
            y_eng.dma_start(out=y_v[:, cb], in_=ob)

    nc.finalize()
    return nc


def _get_nc():
    global _nc_cache
    if _nc_cache is None:
        _nc_cache = _build()
    return _nc_cache


def kernel(x, gamma, _trace=False):
    x = np.ascontiguousarray(np.asarray(x), dtype=np.float32)
    gamma = np.ascontiguousarray(np.asarray(gamma), dtype=np.float32)
    nc = _get_nc()
    in_maps = [
        {"x": np.ascontiguousarray(x[b]), "gamma": gamma} for b in range(B)
    ]
    res = run_bass_kernel_spmd(nc, in_maps, list(range(B)), trace=_trace)
    out = np.stack([r["y"] for r in res.results], axis=0)
    if _trace:
        return out, res
    return out


# revision 21
# speedup vs baseline: 1.2195x; 1.1044x over previous
"""Channel attention (B=8, N=16384, C=512) Trainium2 Bass kernel.

Math (per batch b, with v = x^T [C, N]):
    energy  = v @ v^T                      [C, C]   (gram matrix, symmetric)
    att     = softmax(rowmax(e) - e)       == exp(rowmin(e) - e) / Z  (shift-invariant)
    out     = gamma * (att @ v) + v        [C, N]
    y       = out^T                        [N, C]

Sharding: data-parallel over B — core b computes batch b entirely.

Per-core dataflow (v2 — single fp16 energy matmul, no DRAM scratch):
  Phase 1 (stream x in 128-row chunks, cast to fp16 in-flight by SWDGE DMA):
    energy upper block-triangle += hk_bi^T @ hk_[bi:]  (one fp16 matmul per
    block; quantization error on the final output is ~6e-4, measured off-line)
    hk blocks are also transposed on the PE (regular matmul vs identity
    moving operand — much faster than transpose-mode) into a resident
    SBUF fp16 tensor hiT[c, n] (x^T), so phase 2 needs no DRAM round-trip.
  Interlude:
    mirror the triangle via PE transposes; rowmin; exp(min - e) with fused
    row-sum (ACT accum_out); W = I + gamma/Z * att^T in fp16 via PE matmuls.
    (W folds the softmax normalization, the gamma scale AND the residual.)
  Phase 2:
    y[n-chunk] = hiT-slice^T @ W  accumulated over 4 channel blocks
    (= x @ (I + gamma*att^T) = gamma*(att@v)^T + x, i.e. the final output).
"""

import sys

sys.path.insert(0, "/opt/trn_rl_repo")

from contextlib import ExitStack

import numpy as np

import concourse.bass as bass
import concourse.mybir as mybir
import concourse.tile as tile
from concourse import bacc
from concourse.bass_utils import run_bass_kernel_spmd
from concourse.masks import make_identity

B, N, C = 8, 16384, 512
P = 128
NK = N // P  # 128 row chunks
NB = C // P  # 4 channel blocks
F32 = mybir.dt.float32
F16 = mybir.dt.float16

_nc_cache = None


def _build():
    nc = bacc.Bacc()
    x_in = nc.dram_tensor("x", [N, C], F32, kind="ExternalInput")
    g_in = nc.dram_tensor("gamma", [1], F32, kind="ExternalInput")
    y_out = nc.dram_tensor("y", [N, C], F32, kind="ExternalOutput")

    with ExitStack() as ctx:
        tc = ctx.enter_context(tile.TileContext(nc))
        const = ctx.enter_context(tc.tile_pool(name="const", bufs=1))
        hpool = ctx.enter_context(tc.tile_pool(name="hpool", bufs=4))
        soft = ctx.enter_context(tc.tile_pool(name="soft", bufs=1))
        hiT_pool = ctx.enter_context(tc.tile_pool(name="hiT", bufs=1))
        opool = ctx.enter_context(tc.tile_pool(name="opool", bufs=2))
        psum_t_ctx = tc.tile_pool(name="psum_t", bufs=3, space="PSUM")
        psum_t = psum_t_ctx.__enter__()
        psum_e_ctx = tc.tile_pool(name="psum_e", bufs=1, space="PSUM")
        psum_e = psum_e_ctx.__enter__()

        ident16 = const.tile([P, P], F16)
        make_identity(nc, ident16)
        ident32 = const.tile([P, P], F32)
        make_identity(nc, ident32)
        gamma_sb = const.tile([P, 1], F32)
        nc.sync.dma_start(out=gamma_sb, in_=g_in[:].to_broadcast([P, 1]))

        KB = 4  # k-chunks per phase-1 DMA group
        CB = 4  # n-chunks per phase-2 iteration
        # row n = g*512 + p*4 + u: each partition owns 4 CONSECUTIVE rows of a
        # 512-row group -> 8 KiB contiguous DRAM per partition per DMA (vs 2 KiB
        # with the row-major chunk mapping) => ~340 GB/s instead of ~250.
        # The row permutation is globally consistent: energy sums all rows
        # (order-free), hiT columns are indexed by (g,u)-chunk, and phase 2
        # emits the same permuted rows it reads from hiT.
        x_v = x_in[:].rearrange("(g p u) c -> p g u c", p=P, u=KB)
        y_v = y_out[:].rearrange("(g p u) c -> p g u c", p=P, u=CB)

        # upper-triangle energy accumulators: row-block bi holds cols [bi*P, C)
        e_ps = [psum_e.tile([P, C - bi * P], F32, name=f"e{bi}", tag=f"e{bi}", bufs=1) for bi in range(NB)]
        # resident transposed hi: [c-within-block, block, n] fp16 (= x^T)
        hiT = hiT_pool.tile([P, NB, N], F16, name="hiT", tag="hiT")

        # ---------------- Phase 1: energy + on-chip transpose ----------------
        for kb in range(NK // KB):
            k0 = kb * KB
            hk = hpool.tile([P, KB, C], F16)
            # SWDGE cast-DMA: fp32 DRAM -> fp16 SBUF in flight (8 KiB-contig
            # reads / 4 KiB-contig writes per partition with the permuted view)
            nc.gpsimd.dma_start(out=hk, in_=x_v[:, kb])

            for u in range(KB):
                k = k0 + u
                first = k == 0
                last = k == NK - 1
                tps = psum_t.tile([P, NB, P], F32, name="tps", tag="tps")
                for bi in range(NB):
                    j0 = bi * P
                    lhs = hk[:, u, j0 : j0 + P]
                    nc.tensor.matmul(e_ps[bi], lhs, hk[:, u, j0:C], start=first, stop=last)
                    # transpose of this block: regular matmul, identity moving
                    nc.tensor.matmul(tps[:, bi, :], lhs, ident16, start=True, stop=True)
                if k % 2 == 0:
                    nc.vector.tensor_copy(hiT[:, :, k * P : (k + 1) * P], tps)
                else:
                    nc.scalar.copy(out=hiT[:, :, k * P : (k + 1) * P], in_=tps)

        # ---------------- Interlude: softmax -> W = I + gamma * att^T ----------------
        e_row = [soft.tile([P, C], F32, name=f"erow{bi}", tag=f"erow{bi}") for bi in range(NB)]
        for bi in range(NB):
            if bi % 2 == 0:
                nc.scalar.copy(out=e_row[bi][:, bi * P : C], in_=e_ps[bi])
            else:
                nc.vector.tensor_copy(e_row[bi][:, bi * P : C], e_ps[bi])
        psum_e_ctx.__exit__(None, None, None)
        psum_t_ctx.__exit__(None, None, None)
        psum_m_ctx = tc.tile_pool(name="psum_m", bufs=6, space="PSUM")
        psum_m = psum_m_ctx.__enter__()
        # mirror the strict-lower blocks from the stored upper triangle
        nm = 0
        for bi in range(NB):
            for bj in range(bi):
                pt = psum_m.tile([P, P], F32, tag="tp")
                nc.tensor.transpose(pt, e_row[bj][:, bi * P : (bi + 1) * P], ident32)
                if nm % 2 == 0:
                    nc.vector.tensor_copy(e_row[bi][:, bj * P : (bj + 1) * P], pt)
                else:
                    nc.scalar.copy(out=e_row[bi][:, bj * P : (bj + 1) * P], in_=pt)
                nm += 1

        W = [soft.tile([P, C], F16, name=f"W{bj}", tag=f"W{bj}") for bj in range(NB)]
        Bp = [soft.tile([P, C], F16, name=f"Bp{bi}", tag=f"Bp{bi}") for bi in range(NB)]
        for bi in range(NB):
            mn = soft.tile([P, 1], F32, tag=f"mn{bi}")
            nc.vector.tensor_reduce(
                out=mn, in_=e_row[bi], axis=mybir.AxisListType.X, op=mybir.AluOpType.min
            )
            bt = soft.tile([P, C], F32, tag=f"bt{bi}")
            zt = soft.tile([P, 1], F32, tag=f"zt{bi}")
            nc.scalar.activation(
                out=bt,
                in_=e_row[bi],
                func=mybir.ActivationFunctionType.Exp,
                bias=mn,
                scale=-1.0,
                accum_out=zt,
            )
            rz = soft.tile([P, 1], F32, tag=f"rz{bi}")
            nc.vector.reciprocal(out=rz, in_=zt)
            gr = soft.tile([P, 1], F32, tag=f"gr{bi}")
            nc.vector.tensor_mul(gr, rz, gamma_sb)
            nc.vector.tensor_scalar_mul(Bp[bi], bt, gr)  # fp16: gamma*att rows
        # W[bj][:, bi*P:(bi+1)*P] = Bp[bi][:, bj*P:(bj+1)*P]^T via regular matmuls
        for bi in range(NB):
            for bj in range(NB):
                pw = psum_m.tile([P, P], F32, name="pw", tag="tp")
                nc.tensor.matmul(pw, Bp[bi][:, bj * P : (bj + 1) * P], ident16, start=True, stop=True)
                if (bi + bj) % 2 == 0:
                    nc.vector.tensor_copy(W[bj][:, bi * P : (bi + 1) * P], pw)
                else:
                    nc.scalar.copy(out=W[bj][:, bi * P : (bi + 1) * P], in_=pw)
        for bj in range(NB):
            nc.vector.tensor_add(
                W[bj][:, bj * P : (bj + 1) * P], W[bj][:, bj * P : (bj + 1) * P], ident16
            )

        psum_m_ctx.__exit__(None, None, None)
        psum = ctx.enter_context(tc.tile_pool(name="psum", bufs=2, space="PSUM"))

        # ---------------- Phase 2: y = x @ W ----------------
        for cb in range(NK // CB):
            c0 = cb * CB
            ops = psum.tile([P, CB, C], F32, tag="ops", bufs=2)
            for u in range(CB):
                r0 = (c0 + u) * P
                for bj in range(NB):
                    nc.tensor.matmul(
                        ops[:, u, :],
                        hiT[:, bj, r0 : r0 + P],
                        W[bj],
                        start=(bj == 0),
                        stop=(bj == NB - 1),
                    )
            ob = opool.tile([P, CB, C], F32)
            # split the PSUM drain between ACT and DVE so it hides under the MMs
            nc.scalar.copy(out=ob[:, 0 : CB // 2, :], in_=ops[:, 0 : CB // 2, :])
            nc.vector.tensor_copy(ob[:, CB // 2 : CB, :], ops[:, CB // 2 : CB, :])
            y_eng = nc.sync if cb % 2 == 0 else nc.scalar
            y_eng.dma_start(out=y_v[:, cb], in_=ob)

    nc.finalize()
    return nc


def _get_nc():
    global _nc_cache
    if _nc_cache is None:
        _nc_cache = _build()
    return _nc_cache


def kernel(x, gamma, _trace=False):
    x = np.ascontiguousarray(np.asarray(x), dtype=np.float32)
    gamma = np.ascontiguousarray(np.asarray(gamma), dtype=np.float32)
    nc = _get_nc()
    in_maps = [
        {"x": np.ascontiguousarray(x[b]), "gamma": gamma} for b in range(B)
    ]
    res = run_bass_kernel_spmd(nc, in_maps, list(range(B)), trace=_trace)
    out = np.stack([r["y"] for r in res.results], axis=0)
    if _trace:
        return out, res
    return out
